# revision 23
# baseline (speedup 1.0000x reference)
"""DCT-blur kernel for 8 Trainium2 NeuronCores.

Computes, per image X [256,256]:
    out = C^T @ (M_b (*) (C @ X @ C^T)) @ C
where C is the orthonormal DCT-II matrix and M_b a per-batch-item
frequency fade mask derived from t[b]:
    sigma = exp(log(.5)(1-t) + log(20)t); tau = sigma^2/2
    fade[i,j] = exp(-(f_i^2+f_j^2) tau);  fade<0.01 -> 0
    M = fade*(1-0.001) + 0.001,   f_i = pi*i/256

Since C is orthonormal, C^T D C = X exactly, so
    out = 0.001*X + 0.999 * C^T @ (fade_clamped (*) (C X C^T)) @ C
and fade_clamped has quarter-disk support: when the support fits the
first 128 frequencies ("sparse" batch items, tau large) stages 2-4
shrink from 4+4+4 to 2+1+2 matmuls.

Sharding: pure data parallel, batch 128 -> 16 per core. The host sorts
batch items by frequency cutoff and deals them round-robin so that all
8 cores see the same per-slot sparse/dense pattern (the Bass program is
specialized per slot; one SPMD program for all cores).

Matmul chain (zero explicit transposes; matmul(out,lhsT,rhs) = lhsT.T@rhs,
contracting the partition dim of both operands; stages 2/4 keep the
constant C stationary so their LDWEIGHTS never waits on an eviction):
    S1  = X.T @ C^T          lhsT=X    rhs=C^T  -> [w,k]
    S2T = C_T.T @ S1         lhsT=C^T  rhs=S1   -> [n,k] = (C X C^T)^T
    S2m = S2T (*) mask       (mask symmetric; DVE, during PSUM eviction)
    S3  = S2mT.T @ C         lhsT=S2m  rhs=C    -> [k,h] = S2m C
    Z   = C.T @ S3           lhsT=C    rhs=S3   -> [w,h] rows-on-partitions
    out = Z + 0.001*X        (DVE stt, during PSUM eviction)
All matmuls are [K<=128, M=128, N=256] float32r (full PE rate at N>=256,
measured rel err ~1.4e-4 per 256-length contraction; plain fp32 is exact
but 4 cycles/row -> 2.3x slower end-to-end, switchable via RDT).
Each stage accumulates into a single [128,512] PSUM bank (4 stage tags x
2 bufs = all 8 banks); evictions are single whole-bank ops: S1/S3 on
ACT, S2m-mask-mul and the final stt on DVE.

The fade mask is separable: fade = u (x) u with u[i]=exp(-f_i^2 tau), so
it is built on-device from t: ACT exp for tau and u (per slot, overlapped
with stage 1), K=1 PE outer product, ACT scaled copy (releases the PSUM
bank early), one all-SBUF DVE threshold op.
"""

from contextlib import ExitStack

import numpy as np

import concourse.bass as bass
import concourse.tile as tile
from concourse import bacc, mybir
from concourse.bass_utils import run_bass_kernel_spmd

B, CH, N = 128, 3, 256
NCORES = 8
BPC = B // NCORES  # batch items (slots) per core
IPC = BPC * CH  # images per core
H = N // 2  # 128 = partition count

MIN_BLUR, MAX_BLUR, MIN_SCALE = 0.5, 20.0, 0.001

F32 = mybir.dt.float32
F32R = mybir.dt.float32r
ALU = mybir.AluOpType
ACTF = mybir.ActivationFunctionType


def build_nc(n_b=BPC, flags=None, rdt=F32R):
    """Build the per-core Bass program.

    n_b: batch items (slots) per core; flags[s]=True -> slot s uses the
    sparse (support < 128 frequencies) path. rdt: dtype fed to the PE
    (float32r = fast/reduced precision, float32 = exact/4x slower).
    """
    if flags is None:
        flags = (False,) * n_b
    assert len(flags) == n_b
    n_img = n_b * CH
    nc = bacc.Bacc(
        "TRN2",
        target_bir_lowering=False,
        debug=False,
        num_devices=NCORES,
    )
    x_d = nc.dram_tensor("x", [n_img, 2, H, N], rdt, kind="ExternalInput").ap()
    t_d = nc.dram_tensor("t", [1, n_b], F32, kind="ExternalInput").ap()
    cm_d = nc.dram_tensor("cm", [2, H, N], rdt, kind="ExternalInput").ap()
    cmt_d = nc.dram_tensor("cmt", [2, H, N], rdt, kind="ExternalInput").ap()
    f2_d = nc.dram_tensor("f2", [1, N], F32, kind="ExternalInput").ap()
    y_d = nc.dram_tensor("y", [n_img, 2, H, N], F32, kind="ExternalOutput").ap()

    # tau = sigma^2/2 = exp(ln(1/8) + 2*ln(40)*t)
    TAU_SCALE = float(2.0 * np.log(MAX_BLUR / MIN_BLUR))
    TAU_BIAS = float(np.log(0.5 * MIN_BLUR * MIN_BLUR))

    with tile.TileContext(nc) as tc, ExitStack() as ctx:
        cpool = ctx.enter_context(tc.tile_pool(name="consts", bufs=1))
        wpool = ctx.enter_context(tc.tile_pool(name="work", bufs=3))
        ppool = ctx.enter_context(tc.tile_pool(name="psum", bufs=2, space="PSUM"))

        # ---- constants into SBUF ----
        # cm_sb[:, kk*256:(kk+1)*256] = C[kk*128:(kk+1)*128, :]  (same for C^T)
        cm_sb = cpool.tile([H, 2 * N], rdt, tag="cm", name="cm_sb")
        cmt_sb = cpool.tile([H, 2 * N], rdt, tag="cmt", name="cmt_sb")
        nc.sync.dma_start(
            cm_sb.rearrange("p (k n) -> p k n", k=2), cm_d.rearrange("k p n -> p k n")
        )
        nc.sync.dma_start(
            cmt_sb.rearrange("p (k n) -> p k n", k=2), cmt_d.rearrange("k p n -> p k n")
        )
        f2_sb = cpool.tile([1, N], F32, tag="f2", name="f2_sb")
        nc.sync.dma_start(f2_sb, f2_d)
        t_sb = cpool.tile([1, n_b], F32, tag="t", name="t_sb")
        nc.sync.dma_start(t_sb, t_d)

        # ---- blur schedule: tau then u rows, all on partition 0 ----
        tbias_sb = cpool.tile([1, 1], F32, tag="tbias", name="tbias_sb")
        nc.vector.memset(tbias_sb, TAU_BIAS)
        tau_sb = cpool.tile([1, n_b], F32, tag="tau", name="tau_sb")
        nc.scalar.activation(tau_sb, t_sb, ACTF.Exp, bias=tbias_sb, scale=TAU_SCALE)
        ntau_sb = cpool.tile([1, n_b], F32, tag="ntau", name="ntau_sb")
        nc.vector.tensor_scalar_mul(ntau_sb, tau_sb, -1.0)
        # u_cat[0, b*N + i] = exp(-f_i^2 * tau_b)   (rdt: feeds PE outer prod)
        u_cat = cpool.tile([1, n_b * N], rdt, tag="ucat", name="u_cat")

        mask_sb = {}  # b -> [128, 256 or 512] tile, 0.999*fade_clamped
        # threshold on the 0.999-scaled fade: fade>=0.01 <=> 0.999*fade>=THR
        THR = float(np.float32(np.float32(1.0 - MIN_SCALE) * np.float32(0.01)))

        def build_mask(b, width):
            # mask[k, kk*N + n] = 0.999*clamp(u[k+kk*H]*u[n]) for k-half kk
            nc.scalar.activation(
                u_cat[0:1, b * N : (b + 1) * N],
                f2_sb,
                ACTF.Exp,
                scale=ntau_sb[0:1, b : b + 1],
            )
            psm = ppool.tile([H, width], F32, tag="ps3", name=f"psm_{b}")
            for kk in range(width // N):
                nc.tensor.matmul(
                    psm[:, kk * N : (kk + 1) * N],
                    u_cat[0:1, b * N + kk * H : b * N + kk * H + H],
                    u_cat[0:1, b * N : (b + 1) * N],
                    start=True,
                    stop=True,
                )
            fade = wpool.tile([H, width], F32, tag="fade", name=f"fade_{b}")
            nc.scalar.activation(fade, psm, ACTF.Copy, scale=1.0 - MIN_SCALE)
            m = cpool.tile([H, width], F32, tag=f"mask_{b}", name=f"mask_{b}")
            # m = (fade_s >= THR) * fade_s, single all-SBUF DVE op
            nc.vector.scalar_tensor_tensor(
                m, fade, THR, fade, op0=ALU.is_ge, op1=ALU.mult
            )
            mask_sb[b] = m

        def c_rhs(kk):
            return cm_sb[:, kk * N : (kk + 1) * N]

        def ct_rhs(kk):
            return cmt_sb[:, kk * N : (kk + 1) * N]

        # ---- main loop: triples = the 3 channels of one slot ----
        pending_s4 = []
        for b in range(n_b):
            sparse = flags[b]
            xs, s1, s2, s3 = {}, {}, {}, {}
            for j in range(CH):
                i = b * CH + j
                xf = wpool.tile([H, 2 * N], rdt, tag=f"x{j}", bufs=8, name=f"x_{i}")
                nc.sync.dma_start(
                    xf.rearrange("p (k n) -> p k n", k=2),
                    x_d[i].rearrange("k p n -> p k n"),
                )
                xs[j] = xf
            # stage 1: S1 = X.T @ C^T -> [w, k]; col-block ww = w-half
            for j in range(CH):
                i = b * CH + j
                p1 = ppool.tile([H, 2 * N], F32, tag="ps1", name=f"p1_{i}")
                for m in range(2):
                    for kk in range(2):
                        nc.tensor.matmul(
                            p1[:, m * N : (m + 1) * N],
                            xs[j][:, kk * N + m * H : kk * N + m * H + H],
                            ct_rhs(kk),
                            start=(kk == 0),
                            stop=(kk == 1),
                        )
                s = wpool.tile([H, 2 * N], rdt, tag=f"s1_{j}", name=f"s1_{i}")
                nc.scalar.copy(s, p1)
                s1[j] = s
            if pending_s4:
                pending_s4.pop(0)()
            build_mask(b, N if sparse else 2 * N)
            # stage 2 (C-stationary): S2T = C_T.T @ S1 -> [n, k]; the
            # constant lhsT means no eviction->LDWEIGHTS serialization.
            # Masked eviction (mask is symmetric, layout unchanged).
            n_m2 = 1 if sparse else 2
            for j in range(CH):
                i = b * CH + j
                p2 = ppool.tile([H, n_m2 * N], F32, tag="ps2", name=f"p2_{i}")
                for m in range(n_m2):
                    for ww in range(2):
                        nc.tensor.matmul(
                            p2[:, m * N : (m + 1) * N],
                            cmt_sb[:, ww * N + m * H : ww * N + m * H + H],
                            s1[j][:, ww * N : (ww + 1) * N],
                            start=(ww == 0),
                            stop=(ww == 1),
                        )
                s = wpool.tile([H, n_m2 * N], rdt, tag=f"s2_{j}", name=f"s2_{i}")
                nc.vector.tensor_mul(s, p2, mask_sb[b])
                s2[j] = s
            # stage 3 (data-stationary): S3 = S2mT.T @ C = S2m @ C -> [k, h]
            # sparse: S2m cols k>=128 are all zero -> single k-tile/K-half.
            n_m3 = 1 if sparse else 2
            n_k3 = 1 if sparse else 2
            for j in range(CH):
                i = b * CH + j
                p3 = ppool.tile([H, n_m3 * N], F32, tag="ps3", name=f"p3_{i}")
                for m in range(n_m3):
                    for nn in range(n_k3):
                        nc.tensor.matmul(
                            p3[:, m * N : (m + 1) * N],
                            s2[j][:, nn * N + m * H : nn * N + m * H + H],
                            c_rhs(nn),
                            start=(nn == 0),
                            stop=(nn == n_k3 - 1),
                        )
                s = wpool.tile([H, n_m3 * N], rdt, tag=f"s3_{j}", name=f"s3_{i}")
                nc.scalar.copy(s, p3)
                s3[j] = s
            # stage 4 (C-stationary): Z = C.T @ S3 -> [w, h]; out = Z + 0.001*X
            # Deferred: emitted after the NEXT triple's DMA+stage-1 so the
            # s3-eviction dependency is covered by ~12 matmuls of PE work.
            def emit_s4(b=b, sparse=sparse, xs=xs, s3=s3):
                n_k4 = 1 if sparse else 2
                for j in range(CH):
                    i = b * CH + j
                    p4 = ppool.tile([H, 2 * N], F32, tag="ps4", name=f"p4_{i}")
                    for m in range(2):
                        for kp in range(n_k4):
                            nc.tensor.matmul(
                                p4[:, m * N : (m + 1) * N],
                                cm_sb[:, kp * N + m * H : kp * N + m * H + H],
                                s3[j][:, kp * N : (kp + 1) * N],
                                start=(kp == 0),
                                stop=(kp == n_k4 - 1),
                            )
                    o = wpool.tile([H, 2 * N], F32, tag=f"o{j}", bufs=4, name=f"o_{i}")
                    nc.vector.scalar_tensor_tensor(
                        o, xs[j], MIN_SCALE, p4, op0=ALU.mult, op1=ALU.add
                    )
                    nc.sync.dma_start(
                        y_d[i].rearrange("k p n -> p k n"),
                        o.rearrange("p (k n) -> p k n", k=2),
                    )
            pending_s4.append(emit_s4)
        while pending_s4:
            pending_s4.pop(0)()

    nc.compile()
    return nc


def host_constants():
    n = np.arange(N, dtype=np.float64)
    C = np.cos(np.pi * (n[None, :] + 0.5) * n[:, None] / N)
    scale = np.where(n[:, None] == 0, np.sqrt(1.0 / N), np.sqrt(2.0 / N))
    C = (C * scale).astype(np.float32)
    f = (np.pi * np.arange(N) / N).astype(np.float32)
    f2 = (f * f).astype(np.float32)
    return C, f2


def sparse_of_t(t):
    """True where the clamped fade's support fits the first H freqs (with
    a 2-index safety margin)."""
    t64 = np.asarray(t, dtype=np.float64)
    sigma = np.exp(np.log(MIN_BLUR) * (1 - t64) + np.log(MAX_BLUR) * t64)
    tau = sigma * sigma / 2.0
    lim = np.log(100.0) / tau  # keep (i,j) with f_i^2+f_j^2 <= lim
    f126 = (np.pi * (H - 2) / N) ** 2
    return lim < f126


_CACHE = {}


RDT = F32R  # PE dtype: F32R (fast) or F32 (exact)


def _get_nc(flags):
    key = (flags, RDT)
    if key not in _CACHE:
        _CACHE[key] = build_nc(BPC, flags, rdt=RDT)
    return _CACHE[key]


def _run(x, t, trace=False, tmpdir=None):
    x = np.ascontiguousarray(np.asarray(x, dtype=np.float32))
    t = np.asarray(t, dtype=np.float32)
    assert x.shape == (B, CH, N, N) and t.shape == (B,)

    sparse = sparse_of_t(t)
    # Sort sparse items first so the 8 items of each slot share a flag;
    # deal round-robin: slot s of core c gets sorted item s*8+c.
    order = np.argsort(sparse, kind="stable")  # dense first
    flags = tuple(
        bool(sparse[order[s * NCORES : (s + 1) * NCORES]].all()) for s in range(BPC)
    )
    nc = _get_nc(flags)

    C, f2 = host_constants()
    Cc = np.ascontiguousarray(C)
    Ct = np.ascontiguousarray(C.T)
    in_maps = []
    for c in range(NCORES):
        items = order[np.arange(BPC) * NCORES + c]  # slot s -> batch index
        in_maps.append(
            {
                "x": x[items].reshape(IPC, 2, H, N),
                "t": t[items].reshape(1, BPC),
                "cm": Cc.reshape(2, H, N),
                "cmt": Ct.reshape(2, H, N),
                "f2": f2.reshape(1, N),
            }
        )
    res = run_bass_kernel_spmd(
        nc, in_maps, core_ids=list(range(NCORES)), trace=trace, tmpdir=tmpdir
    )
    out = np.empty_like(x)
    for c in range(NCORES):
        items = order[np.arange(BPC) * NCORES + c]
        out[items] = res.results[c]["y"].reshape(BPC, CH, N, N)
    return out, res


def kernel(x, t):
    out, _ = _run(x, t)
    return out


def kernel_with_profile(x, t, tmpdir=None):
    out, res = _run(x, t, trace=True, tmpdir=tmpdir)
    return out, res


# revision 24
# speedup vs baseline: 1.2110x; 1.2110x over previous
"""DCT-blur kernel for 8 Trainium2 NeuronCores.

Computes, per image X [256,256]:
    out = C^T @ (M_b (*) (C @ X @ C^T)) @ C
where C is the orthonormal DCT-II matrix and M_b a per-batch-item
frequency fade mask derived from t[b]:
    sigma = exp(log(.5)(1-t) + log(20)t); tau = sigma^2/2
    fade[i,j] = exp(-(f_i^2+f_j^2) tau);  fade<0.01 -> 0
    M = fade*(1-0.001) + 0.001,   f_i = pi*i/256

Since C is orthonormal, C^T D C = X exactly, so
    out = 0.001*X + 0.999 * C^T @ (fade_clamped (*) (C X C^T)) @ C
and fade_clamped has quarter-disk support: when the support fits the
first 128 frequencies ("sparse" batch items, tau large) stages 2-4
shrink from 4+4+4 to 2+1+2 matmuls.

Sharding: pure data parallel, batch 128 -> 16 per core. The host sorts
batch items by frequency cutoff and deals them round-robin so that all
8 cores see the same per-slot sparse/dense pattern (the Bass program is
specialized per slot; one SPMD program for all cores).

Matmul chain (zero explicit transposes; matmul(out,lhsT,rhs) = lhsT.T@rhs,
contracting the partition dim of both operands; stages 2/4 keep the
constant C stationary so their LDWEIGHTS never waits on an eviction):
    S1  = X.T @ C^T          lhsT=X    rhs=C^T  -> [w,k]
    S2T = C_T.T @ S1         lhsT=C^T  rhs=S1   -> [n,k] = (C X C^T)^T
    S2m = S2T (*) mask       (mask symmetric; DVE, during PSUM eviction)
    S3  = S2mT.T @ C         lhsT=S2m  rhs=C    -> [k,h] = S2m C
    Z   = C.T @ S3           lhsT=C    rhs=S3   -> [w,h] rows-on-partitions
    out = Z + 0.001*X        (DVE stt, during PSUM eviction)
All matmuls are [K<=128, M=128, N=256] float32r (full PE rate at N>=256,
measured rel err ~1.4e-4 per 256-length contraction; plain fp32 is exact
but 4 cycles/row -> 2.3x slower end-to-end, switchable via RDT).
Each stage accumulates into a single [128,512] PSUM bank (4 stage tags x
2 bufs = all 8 banks); evictions are single whole-bank ops: S1/S3 on
ACT, S2m-mask-mul and the final stt on DVE.

The fade mask is separable: fade = u (x) u with u[i]=exp(-f_i^2 tau), so
it is built on-device from t: ACT exp for tau and u (per slot, overlapped
with stage 1), K=1 PE outer product, ACT scaled copy (releases the PSUM
bank early), one all-SBUF DVE threshold op.
"""

from contextlib import ExitStack

import numpy as np

import concourse.bass as bass
import concourse.tile as tile
from concourse import bacc, mybir
from concourse.bass_utils import run_bass_kernel_spmd

B, CH, N = 128, 3, 256
NCORES = 8
BPC = B // NCORES  # batch items (slots) per core
IPC = BPC * CH  # images per core
H = N // 2  # 128 = partition count

MIN_BLUR, MAX_BLUR, MIN_SCALE = 0.5, 20.0, 0.001

F32 = mybir.dt.float32
F32R = mybir.dt.float32r
ALU = mybir.AluOpType
ACTF = mybir.ActivationFunctionType


def build_nc(n_b=BPC, flags=None, rdt=F32R):
    """Build the per-core Bass program.

    n_b: batch items (slots) per core; flags[s]=True -> slot s uses the
    sparse (support < 128 frequencies) path. rdt: dtype fed to the PE
    (float32r = fast/reduced precision, float32 = exact/4x slower).
    """
    if flags is None:
        flags = (False,) * n_b
    assert len(flags) == n_b
    n_img = n_b * CH
    nc = bacc.Bacc(
        "TRN2",
        target_bir_lowering=False,
        debug=False,
        num_devices=NCORES,
    )
    x_d = nc.dram_tensor("x", [n_img, 2, H, N], rdt, kind="ExternalInput").ap()
    t_d = nc.dram_tensor("t", [1, n_b], F32, kind="ExternalInput").ap()
    cm_d = nc.dram_tensor("cm", [2, H, N], rdt, kind="ExternalInput").ap()
    cmt_d = nc.dram_tensor("cmt", [2, H, N], rdt, kind="ExternalInput").ap()
    f2_d = nc.dram_tensor("f2", [1, N], F32, kind="ExternalInput").ap()
    y_d = nc.dram_tensor("y", [n_img, 2, H, N], F32, kind="ExternalOutput").ap()

    # tau = sigma^2/2 = exp(ln(1/8) + 2*ln(40)*t)
    TAU_SCALE = float(2.0 * np.log(MAX_BLUR / MIN_BLUR))
    TAU_BIAS = float(np.log(0.5 * MIN_BLUR * MIN_BLUR))

    with tile.TileContext(nc) as tc, ExitStack() as ctx:
        cpool = ctx.enter_context(tc.tile_pool(name="consts", bufs=1))
        wpool = ctx.enter_context(tc.tile_pool(name="work", bufs=3))
        ppool = ctx.enter_context(tc.tile_pool(name="psum", bufs=2, space="PSUM"))

        # ---- constants into SBUF ----
        # cm_sb[:, kk*256:(kk+1)*256] = C[kk*128:(kk+1)*128, :]  (same for C^T)
        cm_sb = cpool.tile([H, 2 * N], rdt, tag="cm", name="cm_sb")
        cmt_sb = cpool.tile([H, 2 * N], rdt, tag="cmt", name="cmt_sb")
        nc.sync.dma_start(
            cm_sb.rearrange("p (k n) -> p k n", k=2), cm_d.rearrange("k p n -> p k n")
        )
        nc.sync.dma_start(
            cmt_sb.rearrange("p (k n) -> p k n", k=2), cmt_d.rearrange("k p n -> p k n")
        )
        f2_sb = cpool.tile([1, N], F32, tag="f2", name="f2_sb")
        nc.sync.dma_start(f2_sb, f2_d)
        t_sb = cpool.tile([1, n_b], F32, tag="t", name="t_sb")
        nc.sync.dma_start(t_sb, t_d)

        # ---- blur schedule: tau then u rows, all on partition 0 ----
        tbias_sb = cpool.tile([1, 1], F32, tag="tbias", name="tbias_sb")
        nc.vector.memset(tbias_sb, TAU_BIAS)
        tau_sb = cpool.tile([1, n_b], F32, tag="tau", name="tau_sb")
        nc.scalar.activation(tau_sb, t_sb, ACTF.Exp, bias=tbias_sb, scale=TAU_SCALE)
        ntau_sb = cpool.tile([1, n_b], F32, tag="ntau", name="ntau_sb")
        nc.vector.tensor_scalar_mul(ntau_sb, tau_sb, -1.0)
        # u_cat[0, b*N + i] = exp(-f_i^2 * tau_b)   (rdt: feeds PE outer prod)
        u_cat = cpool.tile([1, n_b * N], rdt, tag="ucat", name="u_cat")

        mask_sb = {}  # b -> [128, 256 or 512] tile, 0.999*fade_clamped
        # threshold on the 0.999-scaled fade: fade>=0.01 <=> 0.999*fade>=THR
        THR = float(np.float32(np.float32(1.0 - MIN_SCALE) * np.float32(0.01)))

        def build_mask(b, width):
            # mask[k, kk*N + n] = 0.999*clamp(u[k+kk*H]*u[n]) for k-half kk
            nc.scalar.activation(
                u_cat[0:1, b * N : (b + 1) * N],
                f2_sb,
                ACTF.Exp,
                scale=ntau_sb[0:1, b : b + 1],
            )
            psm = ppool.tile([H, width], F32, tag="ps3", name=f"psm_{b}")
            for kk in range(width // N):
                nc.tensor.matmul(
                    psm[:, kk * N : (kk + 1) * N],
                    u_cat[0:1, b * N + kk * H : b * N + kk * H + H],
                    u_cat[0:1, b * N : (b + 1) * N],
                    start=True,
                    stop=True,
                )
            fade = wpool.tile([H, width], F32, tag="fade", name=f"fade_{b}")
            nc.scalar.activation(fade, psm, ACTF.Copy, scale=1.0 - MIN_SCALE)
            m = cpool.tile([H, width], F32, tag=f"mask_{b}", name=f"mask_{b}")
            # m = (fade_s >= THR) * fade_s, single all-SBUF DVE op
            nc.vector.scalar_tensor_tensor(
                m, fade, THR, fade, op0=ALU.is_ge, op1=ALU.mult
            )
            mask_sb[b] = m

        def c_rhs(kk):
            return cm_sb[:, kk * N : (kk + 1) * N]

        def ct_rhs(kk):
            return cmt_sb[:, kk * N : (kk + 1) * N]

        # ---- main loop: triples = the 3 channels of one slot ----
        for b in range(n_b):
            sparse = flags[b]
            xs, s1, s2, s3 = {}, {}, {}, {}
            for j in range(CH):
                i = b * CH + j
                xf = wpool.tile([H, 2 * N], rdt, tag=f"x{j}", bufs=8, name=f"x_{i}")
                nc.sync.dma_start(
                    xf.rearrange("p (k n) -> p k n", k=2),
                    x_d[i].rearrange("k p n -> p k n"),
                )
                xs[j] = xf
            # stage 1: S1 = X.T @ C^T -> [w, k]; col-block ww = w-half
            for j in range(CH):
                i = b * CH + j
                p1 = ppool.tile([H, 2 * N], F32, tag="ps1", name=f"p1_{i}")
                for m in range(2):
                    for kk in range(2):
                        nc.tensor.matmul(
                            p1[:, m * N : (m + 1) * N],
                            xs[j][:, kk * N + m * H : kk * N + m * H + H],
                            ct_rhs(kk),
                            start=(kk == 0),
                            stop=(kk == 1),
                        )
                s = wpool.tile([H, 2 * N], rdt, tag=f"s1_{j}", name=f"s1_{i}")
                nc.scalar.copy(s, p1)
                s1[j] = s
            build_mask(b, N if sparse else 2 * N)
            # stage 2 (C-stationary): S2T = C_T.T @ S1 -> [n, k]; the
            # constant lhsT means no eviction->LDWEIGHTS serialization.
            # Masked eviction (mask is symmetric, layout unchanged).
            n_m2 = 1 if sparse else 2
            for j in range(CH):
                i = b * CH + j
                p2 = ppool.tile([H, n_m2 * N], F32, tag="ps2", name=f"p2_{i}")
                for m in range(n_m2):
                    for ww in range(2):
                        nc.tensor.matmul(
                            p2[:, m * N : (m + 1) * N],
                            cmt_sb[:, ww * N + m * H : ww * N + m * H + H],
                            s1[j][:, ww * N : (ww + 1) * N],
                            start=(ww == 0),
                            stop=(ww == 1),
                        )
                s = wpool.tile([H, n_m2 * N], rdt, tag=f"s2_{j}", name=f"s2_{i}")
                nc.vector.tensor_mul(s, p2, mask_sb[b])
                s2[j] = s
            # stage 3 (data-stationary): S3 = S2mT.T @ C = S2m @ C -> [k, h]
            # sparse: S2m cols k>=128 are all zero -> single k-tile/K-half.
            n_m3 = 1 if sparse else 2
            n_k3 = 1 if sparse else 2
            for j in range(CH):
                i = b * CH + j
                p3 = ppool.tile([H, n_m3 * N], F32, tag="ps3", name=f"p3_{i}")
                for m in range(n_m3):
                    for nn in range(n_k3):
                        nc.tensor.matmul(
                            p3[:, m * N : (m + 1) * N],
                            s2[j][:, nn * N + m * H : nn * N + m * H + H],
                            c_rhs(nn),
                            start=(nn == 0),
                            stop=(nn == n_k3 - 1),
                        )
                s = wpool.tile([H, n_m3 * N], rdt, tag=f"s3_{j}", name=f"s3_{i}")
                nc.scalar.copy(s, p3)
                s3[j] = s
            # stage 4 (C-stationary): Z = C.T @ S3 -> [w, h]; out = Z + 0.001*X
            n_k4 = 1 if sparse else 2
            for j in range(CH):
                i = b * CH + j
                p4 = ppool.tile([H, 2 * N], F32, tag="ps4", name=f"p4_{i}")
                for m in range(2):
                    for kp in range(n_k4):
                        nc.tensor.matmul(
                            p4[:, m * N : (m + 1) * N],
                            cm_sb[:, kp * N + m * H : kp * N + m * H + H],
                            s3[j][:, kp * N : (kp + 1) * N],
                            start=(kp == 0),
                            stop=(kp == n_k4 - 1),
                        )
                o = wpool.tile([H, 2 * N], F32, tag=f"o{j}", bufs=4, name=f"o_{i}")
                nc.vector.scalar_tensor_tensor(
                    o, xs[j], MIN_SCALE, p4, op0=ALU.mult, op1=ALU.add
                )
                nc.sync.dma_start(
                    y_d[i].rearrange("k p n -> p k n"),
                    o.rearrange("p (k n) -> p k n", k=2),
                )

    nc.compile()
    return nc


def host_constants():
    n = np.arange(N, dtype=np.float64)
    C = np.cos(np.pi * (n[None, :] + 0.5) * n[:, None] / N)
    scale = np.where(n[:, None] == 0, np.sqrt(1.0 / N), np.sqrt(2.0 / N))
    C = (C * scale).astype(np.float32)
    f = (np.pi * np.arange(N) / N).astype(np.float32)
    f2 = (f * f).astype(np.float32)
    return C, f2


def sparse_of_t(t):
    """True where the clamped fade's support fits the first H freqs (with
    a 2-index safety margin)."""
    t64 = np.asarray(t, dtype=np.float64)
    sigma = np.exp(np.log(MIN_BLUR) * (1 - t64) + np.log(MAX_BLUR) * t64)
    tau = sigma * sigma / 2.0
    lim = np.log(100.0) / tau  # keep (i,j) with f_i^2+f_j^2 <= lim
    f126 = (np.pi * (H - 2) / N) ** 2
    return lim < f126


_CACHE = {}


RDT = F32R  # PE dtype: F32R (fast) or F32 (exact)


def _get_nc(flags):
    key = (flags, RDT)
    if key not in _CACHE:
        _CACHE[key] = build_nc(BPC, flags, rdt=RDT)
    return _CACHE[key]


def _run(x, t, trace=False, tmpdir=None):
    x = np.ascontiguousarray(np.asarray(x, dtype=np.float32))
    t = np.asarray(t, dtype=np.float32)
    assert x.shape == (B, CH, N, N) and t.shape == (B,)

    sparse = sparse_of_t(t)
    # Sort sparse items first so the 8 items of each slot share a flag;
    # deal round-robin: slot s of core c gets sorted item s*8+c.
    order = np.argsort(sparse, kind="stable")  # dense first
    flags = tuple(
        bool(sparse[order[s * NCORES : (s + 1) * NCORES]].all()) for s in range(BPC)
    )
    nc = _get_nc(flags)

    C, f2 = host_constants()
    Cc = np.ascontiguousarray(C)
    Ct = np.ascontiguousarray(C.T)
    in_maps = []
    for c in range(NCORES):
        items = order[np.arange(BPC) * NCORES + c]  # slot s -> batch index
        in_maps.append(
            {
                "x": x[items].reshape(IPC, 2, H, N),
                "t": t[items].reshape(1, BPC),
                "cm": Cc.reshape(2, H, N),
                "cmt": Ct.reshape(2, H, N),
                "f2": f2.reshape(1, N),
            }
        )
    res = run_bass_kernel_spmd(
        nc, in_maps, core_ids=list(range(NCORES)), trace=trace, tmpdir=tmpdir
    )
    out = np.empty_like(x)
    for c in range(NCORES):
        items = order[np.arange(BPC) * NCORES + c]
        out[items] = res.results[c]["y"].reshape(BPC, CH, N, N)
    return out, res


def kernel(x, t):
    out, _ = _run(x, t)
    return out


def kernel_with_profile(x, t, tmpdir=None):
    out, res = _run(x, t, trace=True, tmpdir=tmpdir)
    return out, res
